# revision 43
# baseline (speedup 1.0000x reference)
"""GQA attention block (B=1, T=2048, HID=2048, NQ=16, NKV=8, D=128) on 8 TRN2
NeuronCores.

Sharding: tensor-parallel over heads. Core c owns q-heads {2c, 2c+1} and
kv-head c. The 8 partial [T, HID] outputs (bf16) are summed on the host.

Speed strategy (vs the f32r baseline), validated numerically against the
f64 reference and on the interpreter:
  - projections: 3-term split-fp8 (x = xh+xl; scaled W = wh+wl; terms
    xh*wh + xl*wh + xh*wl) with K=256 DoubleRow matmuls -> ~0.15% error at
    1.33x the f32r rate. Wq/Wk are pre-scaled by 64 so the W-residual
    stays out of fp8 subnormals; the scale cancels through the RMS-norm.
    Wv uses 32 (so |32*v| < fp8e4 max 240) and cancels through a 32-valued
    softmax-denominator vector.
  - q/k stay bf16: RMS-norm + RoPE run on DVE in bf16 at the 2x rate
    (sin is pre-rolled by 64 partitions with the rotate-half sign folded
    in, so RoPE is 4 elementwise ops); scores are plain bf16 matmuls.
  - attention: `at = exp(score/sqrt(D) - 2)` tiles are written plane-
    paired [128, 2, 512] (plane = st parity). For qr>=1 they are fp8, so
    ctx and the denominator sums contract K=256 per DoubleRow matmul; V is
    projected, transposed via the PE into matching [t, d] plane-pairs.
    qr=0 (rows < 512: little softmax averaging to suppress quantization
    noise) uses bf16 `at`/V and plain matmuls. Causal masking uses
    min(at, mask) with mask in {0, 240}: min(inf, 0) = 0, so fp8 overflow
    of exp on (later-masked) acausal scores cannot produce NaN.
  - o_proj: bf16, accumulated over both heads per PSUM chain; the output
    is written bf16 and summed in f32 on the host.
Engine placement and pool depths are tuned against the TimelineSim cost
model (per-10us occupancy profiling): projections/attention keep the PE
~98% busy in phase A; exp (Act), masks (DVE min), and PSUM->bf16
output copies (DVE+Act split) pipeline under the PE in phase C.
"""

import sys

sys.path.insert(0, "/opt/trn_rl_repo")

import numpy as np
import ml_dtypes

import concourse.bass as bass  # noqa: F401  (bass must import before tile)
import concourse.mybir as mybir
import concourse.tile as tile
from concourse import bacc
from concourse.bass_utils import run_bass_kernel_spmd
from concourse.masks import make_identity

N_CORES = 8
T = 2048
HID = 2048
NQ, NKV, D = 16, 8, 128
HQ = NQ // N_CORES  # q heads per core = 2
EPS = 1e-6
SCALE = D**-0.5
SHIFT = 2.0
WS = 64.0  # weight pre-scale for fp8 (q/k: cancels in RMS norm)
VS = 32.0  # v pre-scale: |VS*v| must stay under fp8e4 max 240

P = 128
H = D // 2
KP = HID // 256     # 8 K-pair chunks of 256
NTR = T // 512      # 4 T-ranges of 512
NTT = T // P        # 16 T-tiles of 128

F32 = mybir.dt.float32
F32R = mybir.dt.float32r
BF16 = mybir.dt.bfloat16
F8 = mybir.dt.float8e4
DR = mybir.MatmulPerfMode.DoubleRow
ACT_EXP = mybir.ActivationFunctionType.Exp
ACT_SQRT = mybir.ActivationFunctionType.Sqrt
ACT_SQUARE = mybir.ActivationFunctionType.Square


def build_nc():
    nc = bacc.Bacc("TRN2", target_bir_lowering=False, debug=False,
                   num_devices=N_CORES)

    # ---- DRAM tensors (names = in_map keys) ----
    xh = nc.dram_tensor("xh", [P, KP, 2, T], F8, kind="ExternalInput")
    xl = nc.dram_tensor("xl", [P, KP, 2, T], F8, kind="ExternalInput")
    wqh = nc.dram_tensor("wqh", [P, KP, 2, HQ * D], F8, kind="ExternalInput")
    wql = nc.dram_tensor("wql", [P, KP, 2, HQ * D], F8, kind="ExternalInput")
    wkh = nc.dram_tensor("wkh", [P, KP, 2, D], F8, kind="ExternalInput")
    wkl = nc.dram_tensor("wkl", [P, KP, 2, D], F8, kind="ExternalInput")
    wvh = nc.dram_tensor("wvh", [P, KP, 2, D], F8, kind="ExternalInput")
    wvl = nc.dram_tensor("wvl", [P, KP, 2, D], F8, kind="ExternalInput")
    wob = nc.dram_tensor("wob", [P, HQ, HID], BF16, kind="ExternalInput")
    cosT = nc.dram_tensor("cosT", [P, T], BF16, kind="ExternalInput")
    sinT = nc.dram_tensor("sinT", [P, T], BF16, kind="ExternalInput")
    qw = nc.dram_tensor("qw", [P, 1], F32, kind="ExternalInput")
    kw = nc.dram_tensor("kw", [P, 1], F32, kind="ExternalInput")
    masks = nc.dram_tensor("masks", [P, 4, 512], BF16, kind="ExternalInput")
    out = nc.dram_tensor("out", [T, HID], BF16, kind="ExternalOutput")

    with tile.TileContext(nc) as tc:
        with (
            tc.tile_pool(name="cst", bufs=1) as cst,
            tc.tile_pool(name="fin", bufs=1) as fin,
        ):
            # ---------- constants / weights resident in SBUF ----------
            wqh_sb = cst.tile([P, KP, 2, HQ * D], F8)
            wql_sb = cst.tile([P, KP, 2, HQ * D], F8)
            wkh_sb = cst.tile([P, KP, 2, D], F8)
            wkl_sb = cst.tile([P, KP, 2, D], F8)
            wvh_sb = cst.tile([P, KP, 2, D], F8)
            wvl_sb = cst.tile([P, KP, 2, D], F8)
            wo_sb = cst.tile([P, HQ, HID], BF16)
            masks_sb = cst.tile([P, 4, 512], BF16)
            cos_sb = cst.tile([P, T], BF16)
            sin_sb = cst.tile([P, T], BF16)
            qw_sb = cst.tile([P, 1], F32)
            kw_sb = cst.tile([P, 1], F32)
            nc.scalar.dma_start(qw_sb[:], qw[:])
            nc.scalar.dma_start(kw_sb[:], kw[:])
            ones_r = cst.tile([P, 1], F32R)
            nc.vector.memset(ones_r[:].bitcast(F32), 1.0)
            ones_b = cst.tile([P, 1], BF16)
            nc.vector.memset(ones_b[:], 1.0)
            # DoubleRow ldweights requires the 2-plane dim step % 16 == 0
            w64_8 = cst.tile([P, 2, 16], F8)
            nc.vector.memset(w64_8[:], VS)
            w64_b = cst.tile([P, 1], BF16)
            nc.vector.memset(w64_b[:], VS)
            eps_sb = cst.tile([1, 1], F32)
            nc.vector.memset(eps_sb[:], EPS * WS * WS)
            shift_sb = cst.tile([P, 1], F32)
            nc.vector.memset(shift_sb[:], -SHIFT)
            identf = cst.tile([P, P], F32)
            make_identity(nc, identf[:])
            identb = cst.tile([P, P], BF16)
            nc.gpsimd.tensor_copy(identb[:], identf[:])

            # post RMS+RoPE q/k in bf16 (d on partitions)
            qT = [fin.tile([P, T], BF16, name=f"qT_{s}") for s in range(3)]
            # V (64x): fp8 plane-pairs (plane = st parity) + bf16 st 0-3
            vp = fin.tile([P, NTT // 2, 2, D], F8)
            v0b = fin.tile([P, 4, D], BF16)

            # ==== Phase A (split-fp8 DR projections) + B (RMS+RoPE) ====
            with (
                tc.tile_pool(name="xp", bufs=4) as xp,
                tc.tile_pool(name="vcp", bufs=3) as vcp,
                tc.tile_pool(name="tmpp", bufs=6) as tmpp,
                tc.tile_pool(name="psA", bufs=4, space="PSUM") as psA,
                tc.tile_pool(name="psT", bufs=1, space="PSUM") as psT,
                tc.tile_pool(name="psB", bufs=2, space="PSUM") as psB,
            ):
                for tr in range(NTR):
                    ts = slice(tr * 512, (tr + 1) * 512)
                    xhc = xp.tile([P, KP, 2, 512], F8, name="xhc")
                    xlc = xp.tile([P, KP, 2, 512], F8, name="xlc")
                    if tr == 0:
                        nc.sync.dma_start(xhc[:, 0:4, :, :], xh[:, 0:4, :, ts])
                        nc.sync.dma_start(wqh_sb[:], wqh[:])
                        nc.sync.dma_start(xhc[:, 4:8, :, :], xh[:, 4:8, :, ts])
                        nc.sync.dma_start(wql_sb[:], wql[:])
                    else:
                        nc.sync.dma_start(xhc[:], xh[:, :, :, ts])
                    nc.sync.dma_start(xlc[:], xl[:, :, :, ts])
                    if tr == 0:
                        nc.sync.dma_start(wkh_sb[:], wkh[:])
                        nc.sync.dma_start(wkl_sb[:], wkl[:])
                        nc.sync.dma_start(wvh_sb[:], wvh[:])
                        nc.sync.dma_start(wvl_sb[:], wvl[:])
                        nc.scalar.dma_start(cos_sb[:], cosT[:])
                        nc.scalar.dma_start(sin_sb[:], sinT[:])
                    if tr == 1:
                        nc.scalar.dma_start(masks_sb[:], masks[:])
                        nc.gpsimd.dma_start(wo_sb[:], wob[:])

                    # --- projections q0, q1, k, v (d on partitions) ---
                    raw = []
                    for s in range(4):
                        if s < 2:
                            wh_t, wl_t = wqh_sb, wql_sb
                            cs = slice(s * D, (s + 1) * D)
                        elif s == 2:
                            wh_t, wl_t = wkh_sb, wkl_sb
                            cs = slice(0, D)
                        else:
                            wh_t, wl_t = wvh_sb, wvl_sb
                            cs = slice(0, D)
                        ps = psA.tile([P, 512], F32, name="psA_t")
                        n = 3 * KP
                        i = 0
                        for wt, xt_ in ((wh_t, xhc), (wl_t, xhc), (wh_t, xlc)):
                            for kp in range(KP):
                                nc.tensor.matmul(
                                    ps[:], wt[:, kp, :, cs], xt_[:, kp, :, :],
                                    perf_mode=DR,
                                    start=(i == 0), stop=(i == n - 1),
                                )
                                i += 1
                        raw.append(ps)

                    # --- v: bf16 copy, transpose into [t, d] plane-pairs
                    # (fp8 conversion happens in the plane copy) ---
                    vbc = vcp.tile([P, 512], BF16, name="vbc")
                    nc.scalar.copy(vbc[:], raw[3][:])
                    for j in range(4):
                        st = 4 * tr + j
                        jts = slice(j * P, (j + 1) * P)
                        tpb = psT.tile([P, P], BF16, name="tpb")
                        nc.tensor.transpose(tpb[:], vbc[:, jts], identb[:])
                        nc.vector.tensor_copy(vp[:, st // 2, st % 2, :],
                                              tpb[:])
                        if tr == 0:
                            nc.vector.tensor_copy(v0b[:, st, :], tpb[:])

                    # --- B: RMS norm + RoPE for q0, q1, k (bf16) ---
                    for s in range(3):
                        w_sb = qw_sb if s < 2 else kw_sb
                        src = tmpp.tile([P, 512], F32, name="rawc")
                        nc.scalar.copy(src[:], raw[s][:])
                        sq = tmpp.tile([P, 512], BF16, name="sq")
                        nc.scalar.activation(sq[:], src[:], ACT_SQUARE)
                        ssum = psB.tile([1, 512], F32, name="ssum")
                        nc.tensor.matmul(ssum[:], ones_b[:], sq[:],
                                         start=True, stop=True)
                        # src holds 64*q: 1/sqrt(ssum/D + 64^2 eps) = rinv/64
                        rstd = tmpp.tile([1, 512], F32, name="rstd")
                        nc.scalar.activation(rstd[:], ssum[:], ACT_SQRT,
                                             scale=1.0 / D, bias=eps_sb[:])
                        rinv = tmpp.tile([1, 512], F32, name="rinv")
                        nc.vector.reciprocal_approx_fast(rinv[:], rstd[:])
                        rb = tmpp.tile([P, 512], F32, name="rb")
                        nc.gpsimd.partition_broadcast(rb[:], rinv[:])
                        nq = tmpp.tile([P, 512], BF16, name="nq")
                        nc.vector.scalar_tensor_tensor(
                            nq[:], src[:], w_sb[:], rb[:],
                            mybir.AluOpType.mult, mybir.AluOpType.mult,
                        )
                        # RoPE: sin pre-rolled by 64 partitions with the
                        # rotate-half sign folded in; one full-width add.
                        psn = tmpp.tile([P, 512], BF16, name="psn")
                        nc.vector.tensor_mul(psn[0:H, :], nq[H:D, :],
                                             sin_sb[H:D, ts])
                        nc.vector.tensor_mul(psn[H:D, :], nq[0:H, :],
                                             sin_sb[0:H, ts])
                        pc = tmpp.tile([P, 512], BF16, name="pc")
                        nc.vector.tensor_mul(pc[:], nq[:], cos_sb[:, ts])
                        nc.vector.tensor_add(qT[s][:, ts], pc[:], psn[:])

            # ===== Phase C: attention + o_proj =====
            with (
                tc.tile_pool(name="ctxp", bufs=1) as ctxp,
                tc.tile_pool(name="outp", bufs=3) as outp,
                tc.tile_pool(name="attp", bufs=4) as attp,
                tc.tile_pool(name="atp", bufs=8) as atp,
                tc.tile_pool(name="at0p", bufs=3) as at0p,
                tc.tile_pool(name="psS", bufs=2, space="PSUM") as psS,
                tc.tile_pool(name="psCX", bufs=1, space="PSUM") as psCX,
                tc.tile_pool(name="psSM", bufs=1, space="PSUM") as psSM,
                tc.tile_pool(name="psD", bufs=2, space="PSUM") as psD,
            ):
                ctxT = [ctxp.tile([P, T], BF16, name=f"ctxT{h}")
                        for h in range(HQ)]
                kT = qT[2]

                def emit_oproj(qr):
                    for tt in range(4 * qr, 4 * qr + 4):
                        ot = outp.tile([P, HID], BF16, name="ot")
                        for nr in range(4):
                            ns = slice(nr * 512, (nr + 1) * 512)
                            ps = psD.tile([P, 512], F32, name="psD_t")
                            for h in range(HQ):
                                nc.tensor.matmul(
                                    ps[:],
                                    ctxT[h][:, tt * P:(tt + 1) * P],
                                    wo_sb[:, h, ns],
                                    start=(h == 0), stop=(h == HQ - 1),
                                )
                            osl = ot[:, nr * 512:(nr + 1) * 512]
                            if (tt + nr) % 4 == 3:
                                nc.scalar.copy(osl, ps[:])
                            else:
                                nc.vector.tensor_copy(osl, ps[:])
                            if qr == 3:
                                # tail: don't make the final DMAs wait for
                                # all four copies of the row-block
                                nc.sync.dma_start(
                                    out[tt * P:(tt + 1) * P, ns], osl)
                        if qr < 3:
                            nc.sync.dma_start(out[tt * P:(tt + 1) * P, :],
                                              ot[:])

                for qr in range(NTR):
                    for h in range(HQ):
                        qs = slice(qr * 512, (qr + 1) * 512)
                        n_st = 4 * (qr + 1)
                        n_pair = n_st // 2
                        ctx_ps = psCX.tile([P, 512], F32, name="ctx_ps")
                        sums_ps = psSM.tile([1, 512], F32, name="sums_ps")
                        for pi in range(n_pair):
                            s_ps = psS.tile([P, 2, 512], F32, name="s_ps")
                            for half in range(2):
                                st = 2 * pi + half
                                ks = slice(st * P, (st + 1) * P)
                                nc.tensor.matmul(
                                    s_ps[:, half, :], kT[:, ks],
                                    qT[h][:, qs],
                                    start=True, stop=True)
                            if qr == 0:
                                at = at0p.tile([P, 2, 512], BF16, name="at0")
                            else:
                                at = atp.tile([P, 2, 512], F8, name="at")
                            nc.scalar.activation(at[:], s_ps[:], ACT_EXP,
                                                 scale=SCALE,
                                                 bias=shift_sb[:])
                            for half in range(2):
                                st = 2 * pi + half
                                j = st - 4 * qr
                                if 0 <= j < 4:
                                    # min-mask: at >= 0, and min(inf,0)=0
                                    # avoids inf*0=NaN from acausal-score
                                    # fp8 overflow in exp
                                    nc.vector.tensor_tensor(
                                        at[:, half, :], at[:, half, :],
                                        masks_sb[:, j, :],
                                        mybir.AluOpType.min)
                            # ctx / sums accumulation
                            if qr == 0:
                                for half in range(2):
                                    st = 2 * pi + half
                                    nc.tensor.matmul(
                                        ctx_ps[:], v0b[:, st, :],
                                        at[:, half, :],
                                        start=(st == 0), stop=(st == 3))
                                    nc.tensor.matmul(
                                        sums_ps[:], w64_b[:],
                                        at[:, half, :],
                                        start=(st == 0), stop=(st == 3))
                            else:
                                nc.tensor.matmul(
                                    ctx_ps[:], vp[:, pi, :, :], at[:],
                                    perf_mode=DR,
                                    start=(pi == 0), stop=(pi == n_pair - 1))
                                nc.tensor.matmul(
                                    sums_ps[:], w64_8[:, :, 0:1], at[:],
                                    perf_mode=DR,
                                    start=(pi == 0), stop=(pi == n_pair - 1))
                        recip = attp.tile([1, 512], F32, name="recip")
                        nc.vector.reciprocal_approx_fast(recip[:], sums_ps[:])
                        rb = attp.tile([P, 512], F32, name="rbc")
                        nc.gpsimd.partition_broadcast(rb[:], recip[:])
                        nc.vector.tensor_mul(ctxT[h][:, qs], ctx_ps[:], rb[:])
                        if h == HQ - 1:
                            emit_oproj(qr)

    nc.compile()
    return nc


_NC_CACHE = None


def get_nc():
    global _NC_CACHE
    if _NC_CACHE is None:
        _NC_CACHE = build_nc()
    return _NC_CACHE


F8NP = ml_dtypes.float8_e4m3
BF16NP = ml_dtypes.bfloat16


def _fold_hid(a):
    """[HID, C] -> [P, KP, 2, C] with hid = kp*256 + pl*128 + p."""
    c = a.shape[1]
    return np.ascontiguousarray(
        a.reshape(KP, 2, P, c).transpose(2, 0, 1, 3))


def _split8(a):
    hi = a.astype(F8NP)
    lo = (a - hi.astype(np.float32)).astype(F8NP)
    return hi, lo


def make_in_maps(x, cos, sin, Wq, Wk, Wv, Wo, q_norm_w, k_norm_w):
    x = np.asarray(x, dtype=np.float32).reshape(T, HID)
    xf = _fold_hid(np.ascontiguousarray(x.T).reshape(HID, T))
    xh, xl = _split8(xf)
    cosb = np.ascontiguousarray(
        np.asarray(cos, np.float32).T).astype(BF16NP)
    # rolled by 64 with rotate-half signs folded in:
    # psn[0:64] (subtracted in ref) uses rows 64:128 -> negate those rows
    sr = np.roll(np.asarray(sin, np.float32).T, 64, axis=0)
    sr[64:, :] *= -1.0
    sinb = np.ascontiguousarray(sr).astype(BF16NP)
    qwa = np.ascontiguousarray(
        np.asarray(q_norm_w, np.float32).reshape(D, 1))
    kwa = np.ascontiguousarray(
        np.asarray(k_norm_w, np.float32).reshape(D, 1))
    si = np.arange(P)[:, None, None]
    jj = np.arange(4)[None, :, None]
    qi = np.arange(512)[None, None, :]
    masks = np.where(si + P * jj <= qi, 240.0, 0.0).astype(BF16NP)
    Wq = np.asarray(Wq, np.float32) * WS
    Wk = np.asarray(Wk, np.float32) * WS
    Wv = np.asarray(Wv, np.float32) * VS
    Wo = np.asarray(Wo, np.float32)
    in_maps = []
    for c in range(N_CORES):
        wqh_, wql_ = _split8(_fold_hid(Wq[:, c * HQ * D:(c + 1) * HQ * D]))
        wkh_, wkl_ = _split8(_fold_hid(Wk[:, c * D:(c + 1) * D]))
        wvh_, wvl_ = _split8(_fold_hid(Wv[:, c * D:(c + 1) * D]))
        wo_ = np.ascontiguousarray(
            Wo[c * HQ * D:(c + 1) * HQ * D, :].reshape(HQ, P, HID)
            .transpose(1, 0, 2)).astype(BF16NP)
        in_maps.append({
            "xh": xh, "xl": xl,
            "wqh": wqh_, "wql": wql_,
            "wkh": wkh_, "wkl": wkl_,
            "wvh": wvh_, "wvl": wvl_,
            "wob": wo_,
            "cosT": cosb, "sinT": sinb,
            "qw": qwa, "kw": kwa,
            "masks": masks,
        })
    return in_maps


def kernel(x, cos, sin, Wq, Wk, Wv, Wo, q_norm_w, k_norm_w):
    nc = get_nc()
    in_maps = make_in_maps(x, cos, sin, Wq, Wk, Wv, Wo, q_norm_w, k_norm_w)
    res = run_bass_kernel_spmd(nc, in_maps, core_ids=list(range(N_CORES)))
    acc = np.zeros((T, HID), dtype=np.float32)
    for c in range(N_CORES):
        acc += res.results[c]["out"].astype(np.float32)
    return acc.reshape(1, T, HID)
